# revision 1
# baseline (speedup 1.0000x reference)
"""Trainium2 Bass kernel: AttentionFlow layer (BiDAF-style), data-parallel over batch.

Reference semantics (per batch b, shapes C[Tc,d], Q[Tq,d], w[3d]):
    w1, w2, w3 = w[:d], w[d:2d], w[2d:]
    S[t,q]  = C[t].w1 + Q[q].w2 + (C[t]*w3).Q[q]
    P       = softmax_q(S)
    bt      = softmax_t(max_q S)
    U       = P @ Q
    h       = bt @ C
    G       = concat(C, U, C*U, C*h[None,:])   # [Tc, 4d]

On-chip identities used:
  - softmax_q(S) drops the C.w1 term (constant along q):  P = E/Z with
    E = exp(dot + q2), dot[t,q] = (C*w3)[t].Q[q], q2[q] = Q[q].w2.
    |dot + q2| <~ 5 so exp is fp32-safe without max subtraction.
  - max_q S = c1 + max_q(dot + q2) with c1 = C.w1. The S-matmul rhs gets an
    extra w1 column so c1 lands in column tq of the S psum tile; the q2 row
    is added with a K=1 ones-row matmul. S is only used for the row-max.
  - E^T (for the U matmul) is computed directly as a second matmul
    S'^T = qta^T @ C^T over t-tile PAIRS (output free dim 256 keeps
    float32r matmuls at full rate), then exp'd out of PSUM -- no extra
    S-copy or PE transposes of S.
  - [U_raw | Z] = E @ [Q | 1]  (ones column appended to Q).
  - [h_raw | Zb] = E2^T @ [C | 1] accumulated over t-tiles, E2 = exp(c1+m').
  - Matmuls run as float32r (full-rate fp32 mode, output free >= 256); the
    BIR verifier requires every SBUF operand of an fp32r matmul to be
    PRODUCED as float32r, so all matmul-feeding tiles are allocated f32r
    and non-matmul readers use a plain-f32 bitcast view.
"""

import numpy as np

import concourse.bass as bass
import concourse.bacc as bacc
import concourse.mybir as mybir
import concourse.tile as tile
from contextlib import ExitStack
from concourse.masks import make_identity

F32 = mybir.dt.float32
F32R = mybir.dt.float32r
AX = mybir.AxisListType
AF = mybir.ActivationFunctionType
OP = mybir.AluOpType

B, TC, TQ, D = 32, 2048, 256, 256
N_CORES = 8
BPC = B // N_CORES


def _f32(ap):
    """Plain-fp32 view of a float32r tile for non-matmul readers."""
    return ap.bitcast(F32)


def build_nc(bpc=BPC, tcl=TC, tq=TQ, d=D, reps=None):
    nt = tcl // 128  # t-tiles per batch
    nd = d // 128    # K-chunks over d
    nq = tq // 128   # K-chunks over q
    assert nt % 2 == 0
    cg = min(4, nt)  # t-tiles per C-load DMA group
    ng = nt // cg

    nc = bacc.Bacc(None, debug=False, target_bir_lowering=False)
    c_in = nc.declare_dram_parameter("context_emb", [bpc, tcl, d], F32, isOutput=False)
    q_in = nc.declare_dram_parameter("query_emb", [bpc, tq, d], F32, isOutput=False)
    w_in = nc.declare_dram_parameter("w", [3 * d], F32, isOutput=False)
    out_e = nc.declare_dram_parameter("out", [bpc, tcl, 4 * d], F32, isOutput=True)

    with tile.TileContext(nc) as tc, ExitStack() as ctx:
        singles = ctx.enter_context(tc.tile_pool(name="singles", bufs=1))
        ca_pool = ctx.enter_context(tc.tile_pool(name="ca", bufs=3))
        qb_pool = ctx.enter_context(tc.tile_pool(name="qb", bufs=2))
        pb_pool = ctx.enter_context(tc.tile_pool(name="pb", bufs=2))
        ct_pool = ctx.enter_context(tc.tile_pool(name="ct", bufs=4))
        et_pool = ctx.enter_context(tc.tile_pool(name="et", bufs=4))
        gu_pool = ctx.enter_context(tc.tile_pool(name="gu", bufs=8))
        g4_pool = ctx.enter_context(tc.tile_pool(name="g4", bufs=8))
        sm_pool = ctx.enter_context(tc.tile_pool(name="sm", bufs=6))
        psS = ctx.enter_context(tc.tile_pool(name="psS", bufs=2, space="PSUM"))
        psT = ctx.enter_context(tc.tile_pool(name="psT", bufs=1, space="PSUM"))
        psH = ctx.enter_context(tc.tile_pool(name="psH", bufs=1, space="PSUM"))
        psU = ctx.enter_context(tc.tile_pool(name="psU", bufs=2, space="PSUM"))
        psC = ctx.enter_context(tc.tile_pool(name="psC", bufs=2, space="PSUM"))

        ident = singles.tile([128, 128], F32, tag="ident")
        make_identity(nc, ident)
        # fp32 scratch constants; f32r tiles are produced via copies (memset
        # cannot emit the f32r encoding)
        onesf_col = singles.tile([128, 8], F32, tag="onesf_col")
        nc.vector.memset(onesf_col, 1.0)
        # oz[:, s, :] = [1.0, 0.0] -- pad columns for the even-N f32r matmuls
        oz = singles.tile([128, 8, 2], F32, tag="oz")
        nc.vector.memset(oz[:, :, 0:1], 1.0)
        nc.vector.memset(oz[:, :, 1:2], 0.0)
        zerof_col = singles.tile([128, 1], F32, tag="zerof_col")
        nc.vector.memset(zerof_col, 0.0)
        onesf_row = singles.tile([1, 256], F32, tag="onesf_row")
        nc.vector.memset(onesf_row, 1.0)
        zerof = singles.tile([1, 1], F32, tag="zerof")
        nc.vector.memset(zerof, 0.0)
        ones128 = singles.tile([1, 128], F32R, tag="ones128")
        nc.vector.tensor_copy(out=ones128, in_=onesf_row[:, 0:128])
        ones256 = singles.tile([1, 256], F32R, tag="ones256")
        nc.vector.tensor_copy(out=ones256, in_=onesf_row)
        # wcols[p, k] = w[k*128 + p]: chunk columns [w1 | w2 | w3]
        wcols = singles.tile([128, 3 * nd], F32R, tag="wcols")
        nc.gpsimd.dma_start(
            out=wcols, in_=w_in[:].rearrange("(k p) -> p k", p=128).bitcast(F32R)
        )

        def whole_body(_i=None):
            body()

        def body():
            for b in range(bpc):
                _batch(b)

        def _batch(b):
                # ---- per-batch Q prep ----
                # qaug[:, qi, :] = [Q rows qi*128.. | 1]
                qaug = qb_pool.tile([128, nq, d + 2], F32R, tag="qaug")
                nc.gpsimd.dma_start(
                    out=qaug[:, :, 0:d],
                    in_=q_in[b].rearrange("(s p) d -> p s d", p=128).bitcast(F32R),
                )
                nc.vector.tensor_copy(out=qaug[:, :, d : d + 2], in_=oz[:, 0:nq, :])

                # qt[:, dj, :] = Q^T chunk (d-in-chunk on partitions, q on free)
                qt = qb_pool.tile([128, nd, tq], F32R, tag="qt")
                psq = psC.tile([128, nd * tq], F32, tag="psC")
                for dj in range(nd):
                    for qi in range(nq):
                        nc.tensor.transpose(
                            psq[:, dj * tq + qi * 128 : dj * tq + (qi + 1) * 128],
                            _f32(qaug[:, qi, dj * 128 : (dj + 1) * 128]),
                            ident,
                        )
                nc.scalar.copy(out=qt, in_=psq)

                # q2 row = w2^T @ Q^T -> [1, tq]; pad col tq with 0
                psq2 = psU.tile([1, tq], F32, tag="psU")
                for dj in range(nd):
                    nc.tensor.matmul(
                        psq2,
                        wcols[:, nd + dj : nd + dj + 1],
                        qt[:, dj, :],
                        start=(dj == 0),
                        stop=(dj == nd - 1),
                    )
                q2aug = pb_pool.tile([1, tq + 2], F32R, tag="q2aug")
                nc.vector.tensor_copy(out=q2aug[:, 0:tq], in_=psq2)
                nc.vector.tensor_copy(out=q2aug[:, tq : tq + 2], in_=zerof.to_broadcast([1, 2]))

                # qta[:, dj, :] = [w3-scaled Q^T chunk | w1 chunk column]
                qta = qb_pool.tile([128, nd, tq + 2], F32R, tag="qta")
                for dj in range(nd):
                    nc.vector.tensor_scalar_mul(
                        out=qta[:, dj, 0:tq],
                        in0=_f32(qt[:, dj, :]),
                        scalar1=_f32(wcols[:, 2 * nd + dj : 2 * nd + dj + 1]),
                    )
                    nc.vector.tensor_copy(
                        out=qta[:, dj, tq : tq + 1],
                        in_=_f32(wcols[:, dj : dj + 1]),
                    )
                    nc.vector.tensor_copy(
                        out=qta[:, dj, tq + 1 : tq + 2], in_=zerof_col
                    )

                # ---- load C tiles in groups (resident through phase B) ----
                ca = []
                for g in range(ng):
                    cag = ca_pool.tile([128, cg, d + 2], F32R, tag=f"ca{g}")
                    nc.gpsimd.dma_start(
                        out=cag[:, :, 0:d],
                        in_=c_in[b, g * cg * 128 : (g + 1) * cg * 128, :]
                        .rearrange("(s p) d -> p s d", p=128)
                        .bitcast(F32R),
                    )
                    nc.vector.tensor_copy(out=cag[:, :, d : d + 2], in_=oz[:, 0:cg, :])
                    ca.append(cag)

                def ca_t(j):
                    g, s = divmod(j, cg)
                    return ca[g][:, s, :]

                mfull = pb_pool.tile([128, nt], F32, tag="mfull")
                e2 = pb_pool.tile([128, nt], F32R, tag="e2")
                psh = psH.tile([1, d + 2], F32, tag="psH")

                # ---- phase A: t-tile pairs ----
                for pj in range(nt // 2):
                    # CT for both tiles of the pair: psc2 layout [dj, jj, t]
                    psc2 = psC.tile([128, nd * 256], F32, tag="psC")
                    for jj in range(2):
                        j = 2 * pj + jj
                        for dj in range(nd):
                            nc.tensor.transpose(
                                psc2[:, dj * 256 + jj * 128 : dj * 256 + (jj + 1) * 128],
                                _f32(ca_t(j)[:, dj * 128 : (dj + 1) * 128]),
                                ident,
                            )
                    ct2 = ct_pool.tile([128, nd * 256], F32R, tag="ct2")
                    nc.scalar.copy(out=ct2, in_=psc2)

                    # S[t, q] per tile (only for the row-max) + c1 in col tq
                    for jj in range(2):
                        j = 2 * pj + jj
                        pss = psS.tile([128, tq + 2], F32, tag="psS")
                        for dj in range(nd):
                            nc.tensor.matmul(
                                pss,
                                ct2[:, dj * 256 + jj * 128 : dj * 256 + (jj + 1) * 128],
                                qta[:, dj, :],
                                start=(dj == 0),
                                stop=False,
                            )
                        nc.tensor.matmul(pss, ones128, q2aug, start=False, stop=True)
                        mt = sm_pool.tile([128, 1], F32, tag="mt")
                        nc.vector.reduce_max(out=mt, in_=pss[:, 0:tq], axis=AX.X)
                        nc.vector.tensor_add(
                            out=mfull[:, j : j + 1], in0=mt, in1=pss[:, tq : tq + 1]
                        )

                    # S'^T for the pair: psT2 layout [qi, (jj t)]
                    psT2 = psT.tile([128, nq * 256], F32, tag="psT")
                    for qi in range(nq):
                        sl = slice(qi * 256, (qi + 1) * 256)
                        for dj in range(nd):
                            nc.tensor.matmul(
                                psT2[:, sl],
                                qta[:, dj, qi * 128 : (qi + 1) * 128],
                                ct2[:, dj * 256 : (dj + 1) * 256],
                                start=(dj == 0),
                                stop=False,
                            )
                        nc.tensor.matmul(
                            psT2[:, sl],
                            q2aug[:, qi * 128 : (qi + 1) * 128],
                            ones256,
                            start=False,
                            stop=True,
                        )
                    et2 = et_pool.tile([128, nq * 256], F32R, tag="et2")
                    nc.scalar.activation(out=et2, in_=psT2, func=AF.Exp)

                    # eager bt-softmax numerator + h accumulation for this pair
                    nc.scalar.activation(
                        out=e2[:, 2 * pj : 2 * pj + 2],
                        in_=mfull[:, 2 * pj : 2 * pj + 2],
                        func=AF.Exp,
                    )
                    for jj in range(2):
                        j = 2 * pj + jj
                        nc.tensor.matmul(
                            psh,
                            e2[:, j : j + 1],
                            ca_t(j),
                            start=(j == 0),
                            stop=(j == nt - 1),
                        )

                    # [U_raw | Z] = E @ [Q | 1]; store [C | U | C*U] in phase A
                    for jj in range(2):
                        j = 2 * pj + jj
                        psu = psU.tile([128, d + 2], F32, tag="psU")
                        for qi in range(nq):
                            nc.tensor.matmul(
                                psu,
                                et2[:, qi * 256 + jj * 128 : qi * 256 + (jj + 1) * 128],
                                qaug[:, qi, :],
                                start=(qi == 0),
                                stop=(qi == nq - 1),
                            )
                        rz = sm_pool.tile([128, 1], F32, tag="rz")
                        nc.vector.reciprocal(out=rz, in_=psu[:, d : d + 1])
                        gu = gu_pool.tile([128, 3 * d], F32, tag="gu")
                        nc.gpsimd.tensor_copy(out=gu[:, 0:d], in_=_f32(ca_t(j)[:, 0:d]))
                        nc.scalar.mul(gu[:, d : 2 * d], psu[:, 0:d], rz)
                        nc.vector.tensor_mul(
                            out=gu[:, 2 * d : 3 * d],
                            in0=_f32(ca_t(j)[:, 0:d]),
                            in1=gu[:, d : 2 * d],
                        )
                        (nc.sync if j % 2 == 0 else nc.scalar).dma_start(
                            out=out_e[b, j * 128 : (j + 1) * 128, 0 : 3 * d], in_=gu
                        )

                # ---- phase B: normalize h, then G4 ----
                zb = sm_pool.tile([1, 1], F32, tag="zb")
                nc.vector.reciprocal(out=zb, in_=psh[:, d : d + 1])
                hrow = pb_pool.tile([1, d], F32R, tag="hrow")
                nc.vector.tensor_scalar_mul(out=hrow, in0=psh[:, 0:d], scalar1=zb)
                pshb = psT.tile([128, d], F32, tag="psT")
                nc.tensor.matmul(pshb, ones128, hrow, start=True, stop=True)
                hb = pb_pool.tile([128, d], F32, tag="hb")
                nc.scalar.copy(out=hb, in_=pshb)
                for j in range(nt):
                    g4 = g4_pool.tile([128, d], F32, tag="g4")
                    nc.vector.tensor_mul(out=g4, in0=_f32(ca_t(j)[:, 0:d]), in1=hb)
                    nc.gpsimd.dma_start(
                        out=out_e[b, j * 128 : (j + 1) * 128, 3 * d : 4 * d], in_=g4
                    )


        if reps is None:
            body()
        else:
            with tc.For_i(0, reps, 1):
                body()

    return nc


_NC_CACHE = {}


def _get_nc(bpc=BPC, tcl=TC, tq=TQ, d=D):
    key = (bpc, tcl, tq, d)
    if key not in _NC_CACHE:
        _NC_CACHE[key] = build_nc(*key)
    return _NC_CACHE[key]


def _run(context_emb, query_emb, w, trace=False, **spmd_kwargs):
    from concourse.bass_utils import run_bass_kernel_spmd

    context_emb = np.ascontiguousarray(np.asarray(context_emb, dtype=np.float32))
    query_emb = np.ascontiguousarray(np.asarray(query_emb, dtype=np.float32))
    w = np.ascontiguousarray(np.asarray(w, dtype=np.float32))

    nc = _get_nc()
    if not nc.is_finalized():
        nc.finalize()
    in_maps = []
    for c in range(N_CORES):
        sl = slice(c * BPC, (c + 1) * BPC)
        in_maps.append(
            {
                "context_emb": np.ascontiguousarray(context_emb[sl]),
                "query_emb": np.ascontiguousarray(query_emb[sl]),
                "w": w,
            }
        )
    res = run_bass_kernel_spmd(
        nc, in_maps, core_ids=list(range(N_CORES)), trace=trace, **spmd_kwargs
    )
    out = np.concatenate([r["out"] for r in res.results], axis=0)
    return out, res


def kernel(context_emb, query_emb, w):
    out, _ = _run(context_emb, query_emb, w, trace=False)
    return out

